# revision 32
# baseline (speedup 1.0000x reference)
"""Multi-head attention block (QKV proj + causal-multiplicative-mask softmax
attention + out proj + residual + LayerNorm) on 8 Trainium2 NeuronCores.

Sharding: tensor-parallel over heads for QKV+attention (each core computes 2
of 16 heads), then an AllToAll exchanges the small pre-projection context
(fp8, 128 features per core) so each core applies the full output projection
and residual+LayerNorm on its own contiguous row shard.  This moves 16x less
collective traffic than reduce-scattering the projected partials.

Perf structure:
  - All big matmuls run as fp8e4m3 DoubleRow (2 contraction subtiles per
    instruction, 0.5 cycles/row): QKV projections pair k-chunks, the AV
    matmul pairs consecutive key blocks, the out-projection pairs feature
    chunks.  Scores stay bf16 for accuracy.
  - The multiplicative causal mask (zeros above diagonal -> exp(0)=1) is
    applied by zeroing the diagonal score block in PSUM (one vector op);
    fully-masked key blocks contribute analytically: a ones-column in V
    accumulates the softmax denominator, block column-sums + a static
    selection matrix add all fully-masked suffix blocks.
  - No tensor-engine transposes of attention outputs: the AV result
    [feat, query] is normalized in place (reciprocal + K=1 broadcast
    matmul) and is already the layout the AllToAll payload needs.
  - Weights are scaled x16 (and ctx x8) on the host so fp8 mantissas are
    well used; the inverse scales fold into existing scalar multiplies.
"""

import numpy as np
import ml_dtypes

import concourse.bacc as bacc
import concourse.bass as bass
import concourse.mybir as mybir
import concourse.tile as tile
from concourse.bass_utils import run_bass_kernel_spmd

BF16 = ml_dtypes.bfloat16
E4M3 = ml_dtypes.float8_e4m3
F32 = mybir.dt.float32
BF = mybir.dt.bfloat16
FP8 = mybir.dt.float8e4
FP16 = mybir.dt.float16
FP8E5 = mybir.dt.float8e5
DR = mybir.MatmulPerfMode.DoubleRow

B, S, D = 4, 2048, 1024
H, HD = 16, 64
SCALE = float(HD) ** 0.5
LN_EPS = 1e-5

NCORES = 8
HPC = H // NCORES          # heads per core = 2
FPC = HPC * HD             # feature cols per core = 128
NB = S // 128              # 16 in-batch row blocks of 128
RPB = S // NCORES          # 256 output rows per core per batch

# permutation of a core's 128 q/k feature columns so the projection psum
# partition j = s*64 + h*32 + i holds head h, feature s*32+i (the DoubleRow
# subtile layout needs feature-subtile s at partition i of head-half h)
QKPERM = np.array([(j // 32) % 2 * 64 + (j // 64) * 32 + j % 32
                   for j in range(128)])
WSCALE = 16.0              # host scales W{q,k,v,p} by this before fp8 cast
CSCALE = 8.0               # ctx is scaled by this before fp8 cast (via rcp)
ESCALE = 1.0 / 16.0        # attention weights scaled so exp() fits fp8e4
LOG_ESCALE = float(np.log(ESCALE))

_CACHE = {}


def _build_nc():
    nc = bacc.Bacc("TRN2", target_bir_lowering=False, debug=False,
                   num_devices=NCORES)

    # ---- I/O ----
    xqt = nc.dram_tensor("xqt", [D, B * S], FP8, kind="ExternalInput")
    xkt = nc.dram_tensor("xkt", [D, B * S], FP8, kind="ExternalInput")
    xvt = nc.dram_tensor("xvt", [D, B * S], FP8, kind="ExternalInput")
    wq = nc.dram_tensor("wq", [D, FPC], FP8, kind="ExternalInput")
    wk = nc.dram_tensor("wk", [D, FPC], FP8, kind="ExternalInput")
    wv = nc.dram_tensor("wv", [D, FPC], FP8, kind="ExternalInput")
    wp = nc.dram_tensor("wp", [D, D], FP8, kind="ExternalInput")
    bqs = nc.dram_tensor("bqs", [FPC, 1], F32, kind="ExternalInput")
    bks = nc.dram_tensor("bks", [FPC, 1], F32, kind="ExternalInput")
    bvs = nc.dram_tensor("bvs", [FPC, 1], F32, kind="ExternalInput")
    gam = nc.dram_tensor("gam", [1, D], F32, kind="ExternalInput")
    bet = nc.dram_tensor("bet", [1, D], F32, kind="ExternalInput")
    res = nc.dram_tensor("res", [B * RPB, D], F32, kind="ExternalInput")
    maskt = nc.dram_tensor("maskt", [128, 128], BF, kind="ExternalInput")
    selb = nc.dram_tensor("selb", [NB, NB * 128], FP8, kind="ExternalInput")
    ind16 = nc.dram_tensor("ind16", [128, NB * NB], FP8,
                           kind="ExternalInput")
    ident = nc.dram_tensor("ident", [128, 128], F32, kind="ExternalInput")
    y = nc.dram_tensor("y", [B * RPB, D], F32, kind="ExternalOutput")

    a2a_send = [[nc.dram_tensor(f"a2as{b}h{hb}", [NCORES, 128, 128], FP8)
                 for hb in range(2)] for b in range(B)]
    a2a_recv = [[nc.dram_tensor(f"a2ar{b}h{hb}", [NCORES, 128, 128], FP8)
                 for hb in range(2)] for b in range(B)]

    with tile.TileContext(nc) as tc:
        with tc.tile_pool(name="consts", bufs=1) as cpool, \
                tc.tile_pool(name="xbig", bufs=2) as xbig, \
                tc.tile_pool(name="kv", bufs=3) as kvp, \
                tc.tile_pool(name="exp", bufs=4) as epool, \
                tc.tile_pool(name="ctx", bufs=4) as ctxp, \
                tc.tile_pool(name="norm", bufs=2) as npool, \
                tc.tile_pool(name="cf", bufs=2) as cfpool, \
                tc.tile_pool(name="f32big", bufs=4) as fpool, \
                tc.tile_pool(name="small", bufs=3) as smallp, \
                tc.tile_pool(name="ps_scores", bufs=2, space="PSUM") as spool, \
                tc.tile_pool(name="ps_av", bufs=1, space="PSUM") as apool, \
                tc.tile_pool(name="ps_out", bufs=1, space="PSUM") as opool, \
                tc.tile_pool(name="ps_mm", bufs=2, space="PSUM") as ppool:
            def cload(src, shape, dtype, name):
                t = cpool.tile(shape, dtype, tag=name)
                nc.sync.dma_start(out=t[:], in_=src)
                return t

            # K weights + first x-chunk DMAs go first to shrink the
            # startup stall.  xk is loaded as 8 per-(half, kk-pair) tiles so
            # the first projection matmul starts after 1/8 of the load.
            wk_s = cload(wk.rearrange("(kk p) m -> p kk m", p=128),
                         [128, 8, FPC], FP8, "wks")

            def load_xk(b):
                ts = []
                for hx in range(2):
                    for kp in range(4):
                        t = xbig.tile([128, 2, S // 2], FP8, tag="xk", bufs=16)
                        nc.sync.dma_start(
                            out=t[:],
                            in_=xkt[:, b * S + hx * 1024:
                                    b * S + (hx + 1) * 1024]
                            .rearrange("(kk p) n -> p kk n", p=128)
                            [:, 2 * kp:2 * kp + 2, :])
                        ts.append(t)
                return ts
            xk_first = load_xk(0)

            wv_s = cload(wv.rearrange("(kk p) m -> p kk m", p=128),
                         [128, 8, FPC], FP8, "wvs")
            wq_s = cload(wq.rearrange("(kk p) m -> p kk m", p=128),
                         [128, 8, FPC], FP8, "wqs")
            wp_s = cload(wp.rearrange("(kk p) m -> p kk m", p=128),
                         [128, 8, D], FP8, "wps")
            maskt_s = cload(maskt[:, :], [128, 128], BF, "maskt")
            selb_s = cload(selb[:, :], [NB, NB * 128], FP8, "selb")
            ind16_s = cload(ind16[:, :], [128, NB * NB], FP8, "ind16")
            ident_s = cload(ident[:, :], [128, 128], F32, "ident")
            bq_c = cload(bqs[:, :], [FPC, 1], F32, "bqc")
            bk_c = cload(bks[:, :], [FPC, 1], F32, "bkc")
            bv_c = cload(bvs[:, :], [FPC, 1], F32, "bvc")
            gam_b = cload(gam[:, :].to_broadcast((128, D)), [128, D], F32,
                          "gamb")
            bet_b = cload(bet[:, :].to_broadcast((128, D)), [128, D], F32,
                          "betb")
            # dummy matmuls ramp the PE clock out of its low p-state
            # while the startup DMAs stream
            warm = opool.tile([128, 512], F32, tag="po", name="warm")
            for _ in range(32):
                nc.tensor.matmul(warm[:, 0:128], ident_s[:], ident_s[:],
                                 start=True, stop=True)
            eps_c = cpool.tile([128, 1], F32, tag="eps")
            nc.vector.memset(eps_c[:], LN_EPS)
            ones8 = cpool.tile([1, 64], FP16, tag="ones8")
            nc.vector.memset(ones8[:], CSCALE)
            lesc_c = cpool.tile([128, 1], F32, tag="lesc")
            nc.vector.memset(lesc_c[:], LOG_ESCALE)

            for b in range(B):
                # ---- K projection -> khT [128 feat, 2048 rows] bf16 ----
                xk_h = xk_first if b == 0 else load_xk(b)
                khT = kvp.tile([FPC, S], BF, tag="khT")
                for n in range(4):
                    ps = ppool.tile([128, 512], F32, tag="mm")
                    for kp in range(4):
                        nc.tensor.matmul(
                            ps[:, :], wk_s[:, 2 * kp:2 * kp + 2, :],
                            xk_h[(n // 2) * 4 + kp][
                                :, :, (n % 2) * 512:(n % 2 + 1) * 512],
                            start=(kp == 0), stop=(kp == 3), perf_mode=DR)
                    nc.vector.tensor_scalar(
                        out=khT[:, n * 512:(n + 1) * 512], in0=ps[:, :],
                        scalar1=1.0 / WSCALE, scalar2=bk_c[:, :],
                        op0=mybir.AluOpType.mult, op1=mybir.AluOpType.add)

                # ---- V projection -> vhT f32, transpose, vh65 fp8 ----
                xv_h = []
                for hx in range(2):
                    xvh = xbig.tile([128, 8, S // 2], FP8, tag="xv")
                    nc.sync.dma_start(
                        out=xvh[:],
                        in_=xvt[:, b * S + hx * 1024:b * S + (hx + 1) * 1024]
                        .rearrange("(kk p) n -> p kk n", p=128))
                    xv_h.append(xvh)
                # [part, block-pair, head, block-in-pair, 128]: per-head
                # block pairs contiguous for DoubleRow (M must be 64/128);
                # col 64 = softmax-denominator ones, cols 65..127 zero pad
                vh65 = kvp.tile([128, NB // 2, HPC, 2, 128], FP8, tag="vh65")
                nc.vector.memset(vh65[:, :, :, :, 65:128], 0.0)
                nc.vector.memset(vh65[:, :, :, :, 64:65], 1.0)
                for n in range(4):
                    ps = ppool.tile([128, 512], F32, tag="mm")
                    for kp in range(4):
                        nc.tensor.matmul(
                            ps[:, :], wv_s[:, 2 * kp:2 * kp + 2, :],
                            xv_h[n // 2][:, 2 * kp:2 * kp + 2,
                                         (n % 2) * 512:(n % 2 + 1) * 512],
                            start=(kp == 0), stop=(kp == 3), perf_mode=DR)
                    vhT = fpool.tile([128, 512], F32, tag="vhT", name="vhT")
                    nc.vector.tensor_scalar(
                        out=vhT[:, :], in0=ps[:, :],
                        scalar1=1.0 / WSCALE, scalar2=bv_c[:, :],
                        op0=mybir.AluOpType.mult, op1=mybir.AluOpType.add)
                    for rbl in range(4):
                        rb = n * 4 + rbl
                        pst = ppool.tile([128, 512], F32, tag="mm")
                        nc.tensor.transpose(
                            pst[:, 0:FPC],
                            vhT[:, rbl * 128:(rbl + 1) * 128], ident_s[:])
                        nc.vector.tensor_copy(
                            out=vh65[:, rb // 2, :, rb % 2, 0:64],
                            in_=pst[:, 0:FPC]
                            .rearrange("p (h c) -> p h c", c=64))

                # block column-sums of vh65 (masked-region suffix sums)
                psc = ppool.tile([128, 512], F32, tag="mm")
                for rb in range(NB):
                    nc.tensor.matmul(
                        psc[0:NB, 0:256],
                        ind16_s[:, rb * NB:(rb + 1) * NB],
                        vh65[:, rb // 2, :, rb % 2, :],
                        start=(rb == 0), stop=(rb == NB - 1))
                colsum = kvp.tile([NB, 2, 128], FP8, tag="colsum")
                nc.vector.tensor_scalar(
                    out=colsum[:],
                    in0=psc[0:NB, 0:256].rearrange("b (h c) -> b h c", h=2),
                    scalar1=ESCALE, scalar2=None,
                    op0=mybir.AluOpType.mult)

                # ---- Q projection -> qhT bf16 (1/SCALE folds into the
                # exp activation's scale, keeping q at unit range) ----
                xq_h = []
                for hx in range(2):
                    xqh = xbig.tile([128, 8, S // 2], FP8, tag="xq")
                    nc.sync.dma_start(
                        out=xqh[:],
                        in_=xqt[:, b * S + hx * 1024:b * S + (hx + 1) * 1024]
                        .rearrange("(kk p) n -> p kk n", p=128))
                    xq_h.append(xqh)
                qhT = kvp.tile([FPC, S], BF, tag="qhT")
                for n in range(4):
                    ps = ppool.tile([128, 512], F32, tag="mm")
                    for kp in range(4):
                        nc.tensor.matmul(
                            ps[:, :], wq_s[:, 2 * kp:2 * kp + 2, :],
                            xq_h[n // 2][:, 2 * kp:2 * kp + 2,
                                         (n % 2) * 512:(n % 2 + 1) * 512],
                            start=(kp == 0), stop=(kp == 3), perf_mode=DR)
                    nc.vector.tensor_scalar(
                        out=qhT[:, n * 512:(n + 1) * 512], in0=ps[:, :],
                        scalar1=1.0 / WSCALE, scalar2=bq_c[:, :],
                        op0=mybir.AluOpType.mult, op1=mybir.AluOpType.add)

                # ---- attention, per quad of 4 query blocks ----
                for p in range(NB // 4):
                    qs = 4 * p
                    ng = qs + 4
                    for h01 in range(HPC):
                        hp = slice(h01 * 64, h01 * 64 + 64)
                        av = apool.tile([128, 512], F32, tag="av")
                        for ti in range(ng // 2):
                            g0 = 2 * ti
                            et = epool.tile([128, 1024], FP8E5, tag="et")
                            st = spool.tile([128, 1024], F32, tag="sc")
                            both_full = (g0 + 1) < qs
                            for g in (g0, g0 + 1):
                                col = (g - g0) * 512
                                d = g - qs
                                off = d * 128 if d > 0 else 0
                                nc.tensor.matmul(
                                    st[:, col + off:col + 512],
                                    khT[hp, g * 128:(g + 1) * 128],
                                    qhT[hp, qs * 128 + off:qs * 128 + 512],
                                    start=True, stop=True)
                                if 0 <= d:
                                    # zero masked scores in the diagonal
                                    # block: exp(0)=1 = masked weight
                                    dc = col + d * 128
                                    nc.vector.tensor_mul(
                                        out=st[:, dc:dc + 128],
                                        in0=st[:, dc:dc + 128],
                                        in1=maskt_s[:, :])
                            if both_full:
                                nc.scalar.activation(
                                    out=et[:, :], in_=st[:, :],
                                    func=mybir.ActivationFunctionType.Exp,
                                    bias=lesc_c[:, :], scale=1.0 / SCALE)
                            else:
                                for g in (g0, g0 + 1):
                                    col = (g - g0) * 512
                                    d = g - qs
                                    off = d * 128 if d > 0 else 0
                                    nc.scalar.activation(
                                        out=et[:, col + off:col + 512],
                                        in_=st[:, col + off:col + 512],
                                        func=mybir.ActivationFunctionType.Exp,
                                        bias=lesc_c[:, :], scale=1.0 / SCALE)
                                    if off:
                                        nc.vector.memset(
                                            et[:, col:col + off], ESCALE)
                            nc.tensor.matmul(
                                av[:, :], vh65[:, ti, h01, :, :],
                                et[:].rearrange("p (k n) -> p k n", k=2),
                                start=(ti == 0), stop=False, perf_mode=DR)
                        # suffix sums of fully-masked blocks above the quad
                        nc.tensor.matmul(
                            av[:, :], colsum[:, h01, :],
                            selb_s[:, qs * 128:qs * 128 + 512],
                            start=False, stop=True)
                        # normalize: ctxT = av * (CSCALE / denom)
                        # (copy to a base-0 SBUF row first: custom-DVE ops
                        # mis-handle partition-offset APs)
                        den = npool.tile([1, 512], F32, tag="den")
                        nc.vector.tensor_copy(out=den[:], in_=av[64:65, :])
                        rcpf = npool.tile([1, 512], F32, tag="rcpf")
                        nc.vector.reciprocal_approx_fast(
                            out=rcpf[:], in_=den[:])
                        rcp = npool.tile([1, 512], FP16, tag="rcp")
                        nc.vector.tensor_copy(out=rcp[:], in_=rcpf[:])
                        pb = ppool.tile([128, 512], F32, tag="mm")
                        nc.tensor.matmul(pb[0:64, :], ones8[:], rcp[:],
                                         start=True, stop=True)
                        rb_sb = npool.tile([64, 512], FP16, tag="rbs")
                        nc.vector.tensor_copy(out=rb_sb[:], in_=pb[0:64, :])
                        ctxT = ctxp.tile([64, 512], FP8, tag="ctxT")
                        nc.vector.tensor_mul(out=ctxT[:], in0=av[0:64, :],
                                             in1=rb_sb[:])
                        hb = p // 2        # half-batch this quad is in
                        r0 = 4 * (p % 2)   # first of 4 dest ranks
                        nc.sync.dma_start(
                            out=a2a_send[b][hb][r0:r0 + 4,
                                                64 * h01:64 * h01 + 64, :]
                            .rearrange("r f c -> f r c"),
                            in_=ctxT[:].rearrange("f (r c) -> f r c", r=4))
                    # half-batch AllToAll as soon as quads 0-1 (or 2-3) are
                    # done: out-proj + LN of the matching row block overlap
                    # the rest of the batch
                    if p % 2 == 1:
                        hb = p // 2
                        nc.gpsimd.collective_compute(
                            "AllToAll", mybir.AluOpType.bypass,
                            replica_groups=[list(range(NCORES))],
                            ins=[a2a_send[b][hb][:, :, :].opt()],
                            outs=[a2a_recv[b][hb][:, :, :].opt()])

                # [part, row-block, chunk, 128]: chunk pairs contiguous per
                # row block for the DoubleRow out-projection stationary
                ctxF = cfpool.tile([128, 2, NCORES, 128], FP8, tag="ctxF")
                for hb in range(2):
                    nc.sync.dma_start(
                        out=ctxF[:, hb, :, :],
                        in_=a2a_recv[b][hb].rearrange("c p n -> p c n"))

                # ---- out projection + residual + LayerNorm on own rows ----
                for rb in range(2):
                    rs_t = fpool.tile([128, D], F32, tag="f4k", name="rs_t")
                    nc.sync.dma_start(
                        out=rs_t[:, :],
                        in_=res[b * RPB + rb * 128:b * RPB + (rb + 1) * 128,
                                :])
                    ld = fpool.tile([128, D], F32, tag="f4k", name="ld")
                    for n2 in range(2):
                        pp = opool.tile([128, 512], F32, tag="po")
                        for cp in range(4):
                            nc.tensor.matmul(
                                pp[:, :],
                                ctxF[:, rb, 2 * cp:2 * cp + 2, :],
                                wp_s[:, 2 * cp:2 * cp + 2,
                                     n2 * 512:(n2 + 1) * 512],
                                start=(cp == 0), stop=(cp == 3),
                                perf_mode=DR)
                        nc.vector.scalar_tensor_tensor(
                            out=ld[:, n2 * 512:(n2 + 1) * 512],
                            in0=pp[:, :],
                            scalar=1.0 / (WSCALE * CSCALE),
                            in1=rs_t[:, n2 * 512:(n2 + 1) * 512],
                            op0=mybir.AluOpType.mult,
                            op1=mybir.AluOpType.add)
                    stats = smallp.tile([128, 2, 6], F32, tag="stats",
                                        name="stats")
                    for c2 in range(2):
                        nc.vector.bn_stats(
                            out=stats[:, c2, :],
                            in_=ld[:, c2 * 512:(c2 + 1) * 512])
                    mv = smallp.tile([128, 2], F32, tag="mv", name="mv")
                    nc.vector.bn_aggr(out=mv[:], in_=stats[:])
                    sd = smallp.tile([128, 1], F32, tag="sd", name="sd")
                    nc.scalar.activation(
                        out=sd[:], in_=mv[:, 1:2],
                        func=mybir.ActivationFunctionType.Sqrt,
                        bias=eps_c[:, :])
                    rstd = smallp.tile([128, 1], F32, tag="rstd",
                                       name="rstd")
                    nc.vector.reciprocal(out=rstd[:], in_=sd[:])
                    yt = fpool.tile([128, D], F32, tag="f4k", name="yt")
                    nc.vector.tensor_scalar(
                        out=yt[:, :], in0=ld[:, :],
                        scalar1=mv[:, 0:1], scalar2=rstd[:, :],
                        op0=mybir.AluOpType.subtract,
                        op1=mybir.AluOpType.mult)
                    nc.vector.tensor_mul(out=yt[:, :], in0=yt[:, :],
                                         in1=gam_b[:, :])
                    nc.vector.tensor_add(out=yt[:, :], in0=yt[:, :],
                                         in1=bet_b[:, :])
                    nc.sync.dma_start(
                        out=y[b * RPB + rb * 128:b * RPB + (rb + 1) * 128,
                              :],
                        in_=yt[:, :])

    nc.compile()
    return nc


def _host_inputs(q, k, v, Wq, bq, Wk, bk, Wv, bv, Wp, bp, gamma, beta):
    """Build the 8 per-core input maps from the full-size inputs."""
    qf = np.asarray(q, np.float32).reshape(B * S, D)
    kf = np.asarray(k, np.float32).reshape(B * S, D)
    vf = np.asarray(v, np.float32).reshape(B * S, D)
    xqt = np.ascontiguousarray(qf.T).astype(E4M3)
    xkt = np.ascontiguousarray(kf.T).astype(E4M3)
    xvt = np.ascontiguousarray(vf.T).astype(E4M3)

    Wq = np.asarray(Wq, np.float32) * WSCALE
    Wk = np.asarray(Wk, np.float32) * WSCALE
    Wv = np.asarray(Wv, np.float32) * WSCALE
    Wp = np.asarray(Wp, np.float32) * WSCALE
    bq = np.asarray(bq, np.float32)
    bk = np.asarray(bk, np.float32)
    bv = np.asarray(bv, np.float32)
    bp = np.asarray(bp, np.float32)
    gamma = np.asarray(gamma, np.float32)
    beta = np.asarray(beta, np.float32)

    ii, jj = np.meshgrid(np.arange(128), np.arange(128), indexing="ij")
    maskt = (ii <= jj).astype(BF16)          # [kj, qi]: keep j <= i
    selb = np.zeros((NB, NB * 128), E4M3)
    for p in range(NB // 4):
        selb[4 * p + 4:, p * 512:(p + 1) * 512] = 1
    ind16 = np.zeros((128, NB * NB), E4M3)
    for rb in range(NB):
        ind16[:, rb * NB + rb] = 1
    ident = np.eye(128, dtype=np.float32)

    in_maps = []
    for r in range(NCORES):
        cs = slice(r * FPC, (r + 1) * FPC)
        rows = np.concatenate(
            [np.arange(b * S + hb * (S // 2) + r * 128,
                       b * S + hb * (S // 2) + (r + 1) * 128)
             for b in range(B) for hb in range(2)])
        in_maps.append({
            "xqt": xqt, "xkt": xkt, "xvt": xvt,
            "wq": Wq[:, cs].astype(E4M3),
            "wk": Wk[:, cs].astype(E4M3),
            "wv": Wv[:, cs].astype(E4M3),
            "wp": Wp.astype(E4M3),
            "bqs": bq[cs].reshape(FPC, 1).astype(np.float32),
            "bks": bk[cs].reshape(FPC, 1).astype(np.float32),
            "bvs": bv[cs].reshape(FPC, 1).astype(np.float32),
            "gam": gamma.reshape(1, D),
            "bet": beta.reshape(1, D),
            "res": np.ascontiguousarray(qf[rows] + bp[None, :]),
            "maskt": maskt, "selb": selb, "ind16": ind16, "ident": ident,
        })
    return in_maps


def _assemble(results):
    out = np.empty((B * S, D), np.float32)
    for r in range(NCORES):
        yr = results[r]["y"]
        for b in range(B):
            for hb in range(2):
                g0 = b * S + hb * (S // 2) + r * 128
                l0 = b * RPB + hb * 128
                out[g0:g0 + 128] = yr[l0:l0 + 128]
    return out.reshape(B, S, D)


def kernel(**inputs) -> np.ndarray:
    if "nc" not in _CACHE:
        _CACHE["nc"] = _build_nc()
    nc = _CACHE["nc"]
    in_maps = _host_inputs(**inputs)
    res = run_bass_kernel_spmd(nc, in_maps, core_ids=list(range(NCORES)))
    return _assemble(res.results)


def kernel_profiled(**inputs):
    """Like kernel(), but captures an NTFF profile. Returns (out, result)."""
    if "nc" not in _CACHE:
        _CACHE["nc"] = _build_nc()
    nc = _CACHE["nc"]
    in_maps = _host_inputs(**inputs)
    res = run_bass_kernel_spmd(nc, in_maps, core_ids=list(range(NCORES)),
                               trace=True)
    return _assemble(res.results), res


if __name__ == "__main__":
    rng = np.random.default_rng(0)
    std = 1.0 / np.sqrt(D)
    inp = {
        "q": rng.standard_normal((B, S, D), np.float32),
        "k": rng.standard_normal((B, S, D), np.float32),
        "v": rng.standard_normal((B, S, D), np.float32),
        "Wq": rng.standard_normal((D, D), np.float32) * std,
        "bq": np.zeros(D, np.float32),
        "Wk": rng.standard_normal((D, D), np.float32) * std,
        "bk": np.zeros(D, np.float32),
        "Wv": rng.standard_normal((D, D), np.float32) * std,
        "bv": np.zeros(D, np.float32),
        "Wp": rng.standard_normal((D, D), np.float32) * std,
        "bp": np.zeros(D, np.float32),
        "gamma": np.ones(D, np.float32),
        "beta": np.zeros(D, np.float32),
    }
    out = kernel(**inp)
    print("kernel output:", out.shape, out.dtype)


# revision 33
# speedup vs baseline: 1.0517x; 1.0517x over previous
"""Multi-head attention block (QKV proj + causal-multiplicative-mask softmax
attention + out proj + residual + LayerNorm) on 8 Trainium2 NeuronCores.

Sharding: tensor-parallel over heads for QKV+attention (each core computes 2
of 16 heads), then an AllToAll exchanges the small pre-projection context
(fp8, 128 features per core) so each core applies the full output projection
and residual+LayerNorm on its own contiguous row shard.  This moves 16x less
collective traffic than reduce-scattering the projected partials.

Perf structure:
  - All big matmuls run as fp8e4m3 DoubleRow (2 contraction subtiles per
    instruction, 0.5 cycles/row): QKV projections pair k-chunks, the AV
    matmul pairs consecutive key blocks, the out-projection pairs feature
    chunks.  Scores stay bf16 for accuracy.
  - The multiplicative causal mask (zeros above diagonal -> exp(0)=1) is
    applied by zeroing the diagonal score block in PSUM (one vector op);
    fully-masked key blocks contribute analytically: a ones-column in V
    accumulates the softmax denominator, block column-sums + a static
    selection matrix add all fully-masked suffix blocks.
  - No tensor-engine transposes of attention outputs: the AV result
    [feat, query] is normalized in place (reciprocal + K=1 broadcast
    matmul) and is already the layout the AllToAll payload needs.
  - Weights are scaled x16 (and ctx x8) on the host so fp8 mantissas are
    well used; the inverse scales fold into existing scalar multiplies.
"""

import numpy as np
import ml_dtypes

import concourse.bacc as bacc
import concourse.bass as bass
import concourse.mybir as mybir
import concourse.tile as tile
from concourse.bass_utils import run_bass_kernel_spmd

BF16 = ml_dtypes.bfloat16
E4M3 = ml_dtypes.float8_e4m3
F32 = mybir.dt.float32
BF = mybir.dt.bfloat16
FP8 = mybir.dt.float8e4
FP16 = mybir.dt.float16
FP8E5 = mybir.dt.float8e5
DR = mybir.MatmulPerfMode.DoubleRow

B, S, D = 4, 2048, 1024
H, HD = 16, 64
SCALE = float(HD) ** 0.5
LN_EPS = 1e-5

NCORES = 8
HPC = H // NCORES          # heads per core = 2
FPC = HPC * HD             # feature cols per core = 128
NB = S // 128              # 16 in-batch row blocks of 128
RPB = S // NCORES          # 256 output rows per core per batch

# permutation of a core's 128 q/k feature columns so the projection psum
# partition j = s*64 + h*32 + i holds head h, feature s*32+i (the DoubleRow
# subtile layout needs feature-subtile s at partition i of head-half h)
QKPERM = np.array([(j // 32) % 2 * 64 + (j // 64) * 32 + j % 32
                   for j in range(128)])
WSCALE = 16.0              # host scales W{q,k,v,p} by this before fp8 cast
CSCALE = 8.0               # ctx is scaled by this before fp8 cast (via rcp)
ESCALE = 1.0 / 16.0        # attention weights scaled so exp() fits fp8e4
LOG_ESCALE = float(np.log(ESCALE))

_CACHE = {}


def _build_nc():
    nc = bacc.Bacc("TRN2", target_bir_lowering=False, debug=False,
                   num_devices=NCORES)

    # ---- I/O ----
    xqt = nc.dram_tensor("xqt", [D, B * S], FP8, kind="ExternalInput")
    xkt = nc.dram_tensor("xkt", [D, B * S], FP8, kind="ExternalInput")
    xvt = nc.dram_tensor("xvt", [D, B * S], FP8, kind="ExternalInput")
    wq = nc.dram_tensor("wq", [D, FPC], FP8, kind="ExternalInput")
    wk = nc.dram_tensor("wk", [D, FPC], FP8, kind="ExternalInput")
    wv = nc.dram_tensor("wv", [D, FPC], FP8, kind="ExternalInput")
    wp = nc.dram_tensor("wp", [D, D], FP8, kind="ExternalInput")
    bqs = nc.dram_tensor("bqs", [FPC, 1], F32, kind="ExternalInput")
    bks = nc.dram_tensor("bks", [FPC, 1], F32, kind="ExternalInput")
    bvs = nc.dram_tensor("bvs", [FPC, 1], F32, kind="ExternalInput")
    gam = nc.dram_tensor("gam", [1, D], F32, kind="ExternalInput")
    bet = nc.dram_tensor("bet", [1, D], F32, kind="ExternalInput")
    res = nc.dram_tensor("res", [B * RPB, D], F32, kind="ExternalInput")
    maskt = nc.dram_tensor("maskt", [128, 128], BF, kind="ExternalInput")
    selb = nc.dram_tensor("selb", [NB, NB * 128], FP8, kind="ExternalInput")
    ind16 = nc.dram_tensor("ind16", [128, NB * NB], FP8,
                           kind="ExternalInput")
    ident = nc.dram_tensor("ident", [128, 128], F32, kind="ExternalInput")
    y = nc.dram_tensor("y", [B * RPB, D], F32, kind="ExternalOutput")

    a2a_send = [[nc.dram_tensor(f"a2as{b}h{hb}", [NCORES, 128, 128], FP8)
                 for hb in range(2)] for b in range(B)]
    a2a_recv = [[nc.dram_tensor(f"a2ar{b}h{hb}", [NCORES, 128, 128], FP8)
                 for hb in range(2)] for b in range(B)]

    with tile.TileContext(nc) as tc:
        with tc.tile_pool(name="consts", bufs=1) as cpool, \
                tc.tile_pool(name="xbig", bufs=2) as xbig, \
                tc.tile_pool(name="kv", bufs=3) as kvp, \
                tc.tile_pool(name="exp", bufs=4) as epool, \
                tc.tile_pool(name="ctx", bufs=4) as ctxp, \
                tc.tile_pool(name="norm", bufs=2) as npool, \
                tc.tile_pool(name="cf", bufs=2) as cfpool, \
                tc.tile_pool(name="f32big", bufs=4) as fpool, \
                tc.tile_pool(name="small", bufs=3) as smallp, \
                tc.tile_pool(name="ps_scores", bufs=2, space="PSUM") as spool, \
                tc.tile_pool(name="ps_av", bufs=1, space="PSUM") as apool, \
                tc.tile_pool(name="ps_out", bufs=1, space="PSUM") as opool, \
                tc.tile_pool(name="ps_mm", bufs=2, space="PSUM") as ppool:
            def cload(src, shape, dtype, name, eng=None):
                t = cpool.tile(shape, dtype, tag=name)
                (eng or nc.sync).dma_start(out=t[:], in_=src)
                return t

            # K weights + first x-chunk DMAs go first to shrink the
            # startup stall.  xk is loaded as 8 per-(half, kk-pair) tiles so
            # the first projection matmul starts after 1/8 of the load.
            wk_s = cload(wk.rearrange("(kk p) m -> p kk m", p=128),
                         [128, 8, FPC], FP8, "wks")

            def load_xk(b):
                ts = []
                for hx in range(2):
                    for kp in range(4):
                        t = xbig.tile([128, 2, S // 2], FP8, tag="xk", bufs=16)
                        nc.sync.dma_start(
                            out=t[:],
                            in_=xkt[:, b * S + hx * 1024:
                                    b * S + (hx + 1) * 1024]
                            .rearrange("(kk p) n -> p kk n", p=128)
                            [:, 2 * kp:2 * kp + 2, :])
                        ts.append(t)
                return ts
            xk_first = load_xk(0)

            wv_s = cload(wv.rearrange("(kk p) m -> p kk m", p=128),
                         [128, 8, FPC], FP8, "wvs", eng=nc.scalar)
            wq_s = cload(wq.rearrange("(kk p) m -> p kk m", p=128),
                         [128, 8, FPC], FP8, "wqs", eng=nc.scalar)
            wp_s = cload(wp.rearrange("(kk p) m -> p kk m", p=128),
                         [128, 8, D], FP8, "wps", eng=nc.scalar)
            maskt_s = cload(maskt[:, :], [128, 128], BF, "maskt", eng=nc.scalar)
            selb_s = cload(selb[:, :], [NB, NB * 128], FP8, "selb", eng=nc.scalar)
            ind16_s = cload(ind16[:, :], [128, NB * NB], FP8, "ind16", eng=nc.scalar)
            ident_s = cload(ident[:, :], [128, 128], F32, "ident", eng=nc.scalar)
            bq_c = cload(bqs[:, :], [FPC, 1], F32, "bqc", eng=nc.scalar)
            bk_c = cload(bks[:, :], [FPC, 1], F32, "bkc", eng=nc.scalar)
            bv_c = cload(bvs[:, :], [FPC, 1], F32, "bvc", eng=nc.scalar)
            gam_b = cload(gam[:, :].to_broadcast((128, D)), [128, D], F32,
                          "gamb", eng=nc.scalar)
            bet_b = cload(bet[:, :].to_broadcast((128, D)), [128, D], F32,
                          "betb", eng=nc.scalar)
            # dummy matmuls ramp the PE clock out of its low p-state
            # while the startup DMAs stream
            warm = opool.tile([128, 512], F32, tag="po", name="warm")
            for _ in range(32):
                nc.tensor.matmul(warm[:, 0:128], ident_s[:], ident_s[:],
                                 start=True, stop=True)
            eps_c = cpool.tile([128, 1], F32, tag="eps")
            nc.vector.memset(eps_c[:], LN_EPS)
            ones8 = cpool.tile([1, 64], FP16, tag="ones8")
            nc.vector.memset(ones8[:], CSCALE)
            lesc_c = cpool.tile([128, 1], F32, tag="lesc")
            nc.vector.memset(lesc_c[:], LOG_ESCALE)

            for b in range(B):
                # ---- K projection -> khT [128 feat, 2048 rows] bf16 ----
                xk_h = xk_first if b == 0 else load_xk(b)
                khT = kvp.tile([FPC, S], BF, tag="khT")
                for n in range(4):
                    ps = ppool.tile([128, 512], F32, tag="mm")
                    for kp in range(4):
                        nc.tensor.matmul(
                            ps[:, :], wk_s[:, 2 * kp:2 * kp + 2, :],
                            xk_h[(n // 2) * 4 + kp][
                                :, :, (n % 2) * 512:(n % 2 + 1) * 512],
                            start=(kp == 0), stop=(kp == 3), perf_mode=DR)
                    nc.vector.tensor_scalar(
                        out=khT[:, n * 512:(n + 1) * 512], in0=ps[:, :],
                        scalar1=1.0 / WSCALE, scalar2=bk_c[:, :],
                        op0=mybir.AluOpType.mult, op1=mybir.AluOpType.add)

                # ---- V projection -> vhT f32, transpose, vh65 fp8 ----
                xv_h = []
                for hx in range(2):
                    xvh = xbig.tile([128, 8, S // 2], FP8, tag="xv")
                    nc.sync.dma_start(
                        out=xvh[:],
                        in_=xvt[:, b * S + hx * 1024:b * S + (hx + 1) * 1024]
                        .rearrange("(kk p) n -> p kk n", p=128))
                    xv_h.append(xvh)
                # [part, block-pair, head, block-in-pair, 128]: per-head
                # block pairs contiguous for DoubleRow (M must be 64/128);
                # col 64 = softmax-denominator ones, cols 65..127 zero pad
                vh65 = kvp.tile([128, NB // 2, HPC, 2, 128], FP8, tag="vh65")
                nc.vector.memset(vh65[:, :, :, :, 65:128], 0.0)
                nc.vector.memset(vh65[:, :, :, :, 64:65], 1.0)
                for n in range(4):
                    ps = ppool.tile([128, 512], F32, tag="mm")
                    for kp in range(4):
                        nc.tensor.matmul(
                            ps[:, :], wv_s[:, 2 * kp:2 * kp + 2, :],
                            xv_h[n // 2][:, 2 * kp:2 * kp + 2,
                                         (n % 2) * 512:(n % 2 + 1) * 512],
                            start=(kp == 0), stop=(kp == 3), perf_mode=DR)
                    vhT = fpool.tile([128, 512], F32, tag="vhT", name="vhT")
                    nc.vector.tensor_scalar(
                        out=vhT[:, :], in0=ps[:, :],
                        scalar1=1.0 / WSCALE, scalar2=bv_c[:, :],
                        op0=mybir.AluOpType.mult, op1=mybir.AluOpType.add)
                    for rbl in range(4):
                        rb = n * 4 + rbl
                        pst = ppool.tile([128, 512], F32, tag="mm")
                        nc.tensor.transpose(
                            pst[:, 0:FPC],
                            vhT[:, rbl * 128:(rbl + 1) * 128], ident_s[:])
                        nc.vector.tensor_copy(
                            out=vh65[:, rb // 2, :, rb % 2, 0:64],
                            in_=pst[:, 0:FPC]
                            .rearrange("p (h c) -> p h c", c=64))

                # block column-sums of vh65 (masked-region suffix sums)
                psc = ppool.tile([128, 512], F32, tag="mm")
                for rb in range(NB):
                    nc.tensor.matmul(
                        psc[0:NB, 0:256],
                        ind16_s[:, rb * NB:(rb + 1) * NB],
                        vh65[:, rb // 2, :, rb % 2, :],
                        start=(rb == 0), stop=(rb == NB - 1))
                colsum = kvp.tile([NB, 2, 128], FP8, tag="colsum")
                nc.vector.tensor_scalar(
                    out=colsum[:],
                    in0=psc[0:NB, 0:256].rearrange("b (h c) -> b h c", h=2),
                    scalar1=ESCALE, scalar2=None,
                    op0=mybir.AluOpType.mult)

                # ---- Q projection -> qhT bf16 (1/SCALE folds into the
                # exp activation's scale, keeping q at unit range) ----
                xq_h = []
                for hx in range(2):
                    xqh = xbig.tile([128, 8, S // 2], FP8, tag="xq")
                    nc.sync.dma_start(
                        out=xqh[:],
                        in_=xqt[:, b * S + hx * 1024:b * S + (hx + 1) * 1024]
                        .rearrange("(kk p) n -> p kk n", p=128))
                    xq_h.append(xqh)
                qhT = kvp.tile([FPC, S], BF, tag="qhT")
                for n in range(4):
                    ps = ppool.tile([128, 512], F32, tag="mm")
                    for kp in range(4):
                        nc.tensor.matmul(
                            ps[:, :], wq_s[:, 2 * kp:2 * kp + 2, :],
                            xq_h[n // 2][:, 2 * kp:2 * kp + 2,
                                         (n % 2) * 512:(n % 2 + 1) * 512],
                            start=(kp == 0), stop=(kp == 3), perf_mode=DR)
                    nc.vector.tensor_scalar(
                        out=qhT[:, n * 512:(n + 1) * 512], in0=ps[:, :],
                        scalar1=1.0 / WSCALE, scalar2=bq_c[:, :],
                        op0=mybir.AluOpType.mult, op1=mybir.AluOpType.add)

                # ---- attention, per quad of 4 query blocks ----
                for p in range(NB // 4):
                    qs = 4 * p
                    ng = qs + 4
                    for h01 in range(HPC):
                        hp = slice(h01 * 64, h01 * 64 + 64)
                        av = apool.tile([128, 512], F32, tag="av")
                        for ti in range(ng // 2):
                            g0 = 2 * ti
                            et = epool.tile([128, 1024], FP8E5, tag="et")
                            st = spool.tile([128, 1024], F32, tag="sc")
                            both_full = (g0 + 1) < qs
                            for g in (g0, g0 + 1):
                                col = (g - g0) * 512
                                d = g - qs
                                off = d * 128 if d > 0 else 0
                                nc.tensor.matmul(
                                    st[:, col + off:col + 512],
                                    khT[hp, g * 128:(g + 1) * 128],
                                    qhT[hp, qs * 128 + off:qs * 128 + 512],
                                    start=True, stop=True)
                                if 0 <= d:
                                    # zero masked scores in the diagonal
                                    # block: exp(0)=1 = masked weight
                                    dc = col + d * 128
                                    nc.vector.tensor_mul(
                                        out=st[:, dc:dc + 128],
                                        in0=st[:, dc:dc + 128],
                                        in1=maskt_s[:, :])
                            if both_full:
                                nc.scalar.activation(
                                    out=et[:, :], in_=st[:, :],
                                    func=mybir.ActivationFunctionType.Exp,
                                    bias=lesc_c[:, :], scale=1.0 / SCALE)
                            else:
                                for g in (g0, g0 + 1):
                                    col = (g - g0) * 512
                                    d = g - qs
                                    off = d * 128 if d > 0 else 0
                                    nc.scalar.activation(
                                        out=et[:, col + off:col + 512],
                                        in_=st[:, col + off:col + 512],
                                        func=mybir.ActivationFunctionType.Exp,
                                        bias=lesc_c[:, :], scale=1.0 / SCALE)
                                    if off:
                                        nc.vector.memset(
                                            et[:, col:col + off], ESCALE)
                            nc.tensor.matmul(
                                av[:, :], vh65[:, ti, h01, :, :],
                                et[:].rearrange("p (k n) -> p k n", k=2),
                                start=(ti == 0), stop=False, perf_mode=DR)
                        # suffix sums of fully-masked blocks above the quad
                        nc.tensor.matmul(
                            av[:, :], colsum[:, h01, :],
                            selb_s[:, qs * 128:qs * 128 + 512],
                            start=False, stop=True)
                        # normalize: ctxT = av * (CSCALE / denom)
                        # (copy to a base-0 SBUF row first: custom-DVE ops
                        # mis-handle partition-offset APs)
                        den = npool.tile([1, 512], F32, tag="den")
                        nc.vector.tensor_copy(out=den[:], in_=av[64:65, :])
                        rcpf = npool.tile([1, 512], F32, tag="rcpf")
                        nc.vector.reciprocal_approx_fast(
                            out=rcpf[:], in_=den[:])
                        rcp = npool.tile([1, 512], FP16, tag="rcp")
                        nc.vector.tensor_copy(out=rcp[:], in_=rcpf[:])
                        pb = ppool.tile([128, 512], F32, tag="mm")
                        nc.tensor.matmul(pb[0:64, :], ones8[:], rcp[:],
                                         start=True, stop=True)
                        rb_sb = npool.tile([64, 512], FP16, tag="rbs")
                        nc.vector.tensor_copy(out=rb_sb[:], in_=pb[0:64, :])
                        ctxT = ctxp.tile([64, 512], FP8, tag="ctxT")
                        nc.vector.tensor_mul(out=ctxT[:], in0=av[0:64, :],
                                             in1=rb_sb[:])
                        hb = p // 2        # half-batch this quad is in
                        r0 = 4 * (p % 2)   # first of 4 dest ranks
                        nc.sync.dma_start(
                            out=a2a_send[b][hb][r0:r0 + 4,
                                                64 * h01:64 * h01 + 64, :]
                            .rearrange("r f c -> f r c"),
                            in_=ctxT[:].rearrange("f (r c) -> f r c", r=4))
                    # half-batch AllToAll as soon as quads 0-1 (or 2-3) are
                    # done: out-proj + LN of the matching row block overlap
                    # the rest of the batch
                    if p % 2 == 1:
                        hb = p // 2
                        nc.gpsimd.collective_compute(
                            "AllToAll", mybir.AluOpType.bypass,
                            replica_groups=[list(range(NCORES))],
                            ins=[a2a_send[b][hb][:, :, :].opt()],
                            outs=[a2a_recv[b][hb][:, :, :].opt()])

                # [part, row-block, chunk, 128]: chunk pairs contiguous per
                # row block for the DoubleRow out-projection stationary
                ctxF = cfpool.tile([128, 2, NCORES, 128], FP8, tag="ctxF")
                for hb in range(2):
                    nc.sync.dma_start(
                        out=ctxF[:, hb, :, :],
                        in_=a2a_recv[b][hb].rearrange("c p n -> p c n"))

                # ---- out projection + residual + LayerNorm on own rows ----
                for rb in range(2):
                    rs_t = fpool.tile([128, D], F32, tag="f4k", name="rs_t")
                    nc.sync.dma_start(
                        out=rs_t[:, :],
                        in_=res[b * RPB + rb * 128:b * RPB + (rb + 1) * 128,
                                :])
                    ld = fpool.tile([128, D], F32, tag="f4k", name="ld")
                    for n2 in range(2):
                        pp = opool.tile([128, 512], F32, tag="po")
                        for cp in range(4):
                            nc.tensor.matmul(
                                pp[:, :],
                                ctxF[:, rb, 2 * cp:2 * cp + 2, :],
                                wp_s[:, 2 * cp:2 * cp + 2,
                                     n2 * 512:(n2 + 1) * 512],
                                start=(cp == 0), stop=(cp == 3),
                                perf_mode=DR)
                        nc.vector.scalar_tensor_tensor(
                            out=ld[:, n2 * 512:(n2 + 1) * 512],
                            in0=pp[:, :],
                            scalar=1.0 / (WSCALE * CSCALE),
                            in1=rs_t[:, n2 * 512:(n2 + 1) * 512],
                            op0=mybir.AluOpType.mult,
                            op1=mybir.AluOpType.add)
                    stats = smallp.tile([128, 2, 6], F32, tag="stats",
                                        name="stats")
                    for c2 in range(2):
                        nc.vector.bn_stats(
                            out=stats[:, c2, :],
                            in_=ld[:, c2 * 512:(c2 + 1) * 512])
                    mv = smallp.tile([128, 2], F32, tag="mv", name="mv")
                    nc.vector.bn_aggr(out=mv[:], in_=stats[:])
                    sd = smallp.tile([128, 1], F32, tag="sd", name="sd")
                    nc.scalar.activation(
                        out=sd[:], in_=mv[:, 1:2],
                        func=mybir.ActivationFunctionType.Sqrt,
                        bias=eps_c[:, :])
                    rstd = smallp.tile([128, 1], F32, tag="rstd",
                                       name="rstd")
                    nc.vector.reciprocal(out=rstd[:], in_=sd[:])
                    yt = fpool.tile([128, D], F32, tag="f4k", name="yt")
                    nc.vector.tensor_scalar(
                        out=yt[:, :], in0=ld[:, :],
                        scalar1=mv[:, 0:1], scalar2=rstd[:, :],
                        op0=mybir.AluOpType.subtract,
                        op1=mybir.AluOpType.mult)
                    nc.vector.tensor_mul(out=yt[:, :], in0=yt[:, :],
                                         in1=gam_b[:, :])
                    nc.vector.tensor_add(out=yt[:, :], in0=yt[:, :],
                                         in1=bet_b[:, :])
                    nc.sync.dma_start(
                        out=y[b * RPB + rb * 128:b * RPB + (rb + 1) * 128,
                              :],
                        in_=yt[:, :])

    nc.compile()
    return nc


def _host_inputs(q, k, v, Wq, bq, Wk, bk, Wv, bv, Wp, bp, gamma, beta):
    """Build the 8 per-core input maps from the full-size inputs."""
    qf = np.asarray(q, np.float32).reshape(B * S, D)
    kf = np.asarray(k, np.float32).reshape(B * S, D)
    vf = np.asarray(v, np.float32).reshape(B * S, D)
    xqt = np.ascontiguousarray(qf.T).astype(E4M3)
    xkt = np.ascontiguousarray(kf.T).astype(E4M3)
    xvt = np.ascontiguousarray(vf.T).astype(E4M3)

    Wq = np.asarray(Wq, np.float32) * WSCALE
    Wk = np.asarray(Wk, np.float32) * WSCALE
    Wv = np.asarray(Wv, np.float32) * WSCALE
    Wp = np.asarray(Wp, np.float32) * WSCALE
    bq = np.asarray(bq, np.float32)
    bk = np.asarray(bk, np.float32)
    bv = np.asarray(bv, np.float32)
    bp = np.asarray(bp, np.float32)
    gamma = np.asarray(gamma, np.float32)
    beta = np.asarray(beta, np.float32)

    ii, jj = np.meshgrid(np.arange(128), np.arange(128), indexing="ij")
    maskt = (ii <= jj).astype(BF16)          # [kj, qi]: keep j <= i
    selb = np.zeros((NB, NB * 128), E4M3)
    for p in range(NB // 4):
        selb[4 * p + 4:, p * 512:(p + 1) * 512] = 1
    ind16 = np.zeros((128, NB * NB), E4M3)
    for rb in range(NB):
        ind16[:, rb * NB + rb] = 1
    ident = np.eye(128, dtype=np.float32)

    in_maps = []
    for r in range(NCORES):
        cs = slice(r * FPC, (r + 1) * FPC)
        rows = np.concatenate(
            [np.arange(b * S + hb * (S // 2) + r * 128,
                       b * S + hb * (S // 2) + (r + 1) * 128)
             for b in range(B) for hb in range(2)])
        in_maps.append({
            "xqt": xqt, "xkt": xkt, "xvt": xvt,
            "wq": Wq[:, cs].astype(E4M3),
            "wk": Wk[:, cs].astype(E4M3),
            "wv": Wv[:, cs].astype(E4M3),
            "wp": Wp.astype(E4M3),
            "bqs": bq[cs].reshape(FPC, 1).astype(np.float32),
            "bks": bk[cs].reshape(FPC, 1).astype(np.float32),
            "bvs": bv[cs].reshape(FPC, 1).astype(np.float32),
            "gam": gamma.reshape(1, D),
            "bet": beta.reshape(1, D),
            "res": np.ascontiguousarray(qf[rows] + bp[None, :]),
            "maskt": maskt, "selb": selb, "ind16": ind16, "ident": ident,
        })
    return in_maps


def _assemble(results):
    out = np.empty((B * S, D), np.float32)
    for r in range(NCORES):
        yr = results[r]["y"]
        for b in range(B):
            for hb in range(2):
                g0 = b * S + hb * (S // 2) + r * 128
                l0 = b * RPB + hb * 128
                out[g0:g0 + 128] = yr[l0:l0 + 128]
    return out.reshape(B, S, D)


def kernel(**inputs) -> np.ndarray:
    if "nc" not in _CACHE:
        _CACHE["nc"] = _build_nc()
    nc = _CACHE["nc"]
    in_maps = _host_inputs(**inputs)
    res = run_bass_kernel_spmd(nc, in_maps, core_ids=list(range(NCORES)))
    return _assemble(res.results)


def kernel_profiled(**inputs):
    """Like kernel(), but captures an NTFF profile. Returns (out, result)."""
    if "nc" not in _CACHE:
        _CACHE["nc"] = _build_nc()
    nc = _CACHE["nc"]
    in_maps = _host_inputs(**inputs)
    res = run_bass_kernel_spmd(nc, in_maps, core_ids=list(range(NCORES)),
                               trace=True)
    return _assemble(res.results), res


if __name__ == "__main__":
    rng = np.random.default_rng(0)
    std = 1.0 / np.sqrt(D)
    inp = {
        "q": rng.standard_normal((B, S, D), np.float32),
        "k": rng.standard_normal((B, S, D), np.float32),
        "v": rng.standard_normal((B, S, D), np.float32),
        "Wq": rng.standard_normal((D, D), np.float32) * std,
        "bq": np.zeros(D, np.float32),
        "Wk": rng.standard_normal((D, D), np.float32) * std,
        "bk": np.zeros(D, np.float32),
        "Wv": rng.standard_normal((D, D), np.float32) * std,
        "bv": np.zeros(D, np.float32),
        "Wp": rng.standard_normal((D, D), np.float32) * std,
        "bp": np.zeros(D, np.float32),
        "gamma": np.ones(D, np.float32),
        "beta": np.zeros(D, np.float32),
    }
    out = kernel(**inp)
    print("kernel output:", out.shape, out.dtype)
